# revision 3
# baseline (speedup 1.0000x reference)
"""Multi-head causal attention (B=2, S=2048, E=1024, H=16, D=64) on 8 TRN2
NeuronCores. Sharding: batch (2) x head-groups (4 heads each) -> 8 cores.
Each core computes Q/K/V projections for its 4 heads, RoPE, flash-style
causal attention, and a partial output projection (its head columns of Wo);
the host sums the 4 partials per batch.

Layout notes:
- Q/K projections run in fp8e4 DoubleRow (x fp8 at unit scale, W fp8
  pre-scaled by 32; the 1/32 is folded into the RoPE cos/sin tables), one
  DR matmul per e-chunk-pair: 2x the bf16 PE throughput. V projection and
  the output projection stay bf16 for accuracy. fp8 bytes DMA as uint8 and
  are bitcast at use (axon PJRT lacks fp8 buffers).
- Per head, dims are permuted to [evens(32); odds(32)]; RoPE is 6
  full-width [128,512] DVE ops per S-chunk, and the rotated q/k quantize
  to one [128, 2, 512] fp8 tile per chunk (dim1 = even/odd).
- Scores are one full-array fp8 DoubleRow matmul per (head, k-block): the
  stationary is a zero-padded per-head copy of K ([128,2,128] with only
  rows 32h..32h+31 nonzero, built by DMA on the Activation HWDGE queue),
  the even/odd halves riding the DR 2-ktile dim. This avoids the ~2.3x
  per-column penalty of sub-128-row (tile_position) matmuls and keeps the
  PE in one tiling mode throughout.
- Scores are computed transposed (scoresT[k,q]) so the AV matmul can use
  V as the stationary operand in natural [S, dims] layout; a ones column
  appended to V yields the softmax denominators in psum row 64.
- Softmax normalization: reciprocal (DVE) -> partition_broadcast (GPSIMD,
  attn ucode library) -> multiply during the psum->SBUF drain (DVE).
- The attention i-loop is software-pipelined: scores(i+1) is emitted
  before exp/AV(i) so the in-order PE queue runs scores during the scalar
  engine's exp instead of stalling behind AV's wait. The scalar engine
  (exp: ~450ns/op + ~0.63ns/col from psum) is the binding engine at
  ~80us/core; PE work is ~80us/core.
- Output partials are written bf16 (halves the out-DMA and doubles the
  DVE drain rate); the host sums partials in fp32. Measured ~110-120us/core
  on TRN2 silicon (reps=9 in-NEFF repetition, marginal/8), rel err ~1.6e-2
  (fp8 q/k quantization dominates; gate is 2e-2).
"""

import sys

if "/opt/trn_rl_repo" not in sys.path:
    sys.path.insert(0, "/opt/trn_rl_repo")

import numpy as np
import ml_dtypes

B, S, E, H = 2, 2048, 1024, 16
D = E // H          # 64
HPC = 4             # heads per core
NCORES = 8
NE = E // 128       # 8 contraction chunks
NQ = S // 512       # 4 q-chunks
NK = S // 128       # 16 k-blocks
ROPE_BASE = 10000.0
ATTN_SCALE = 1.0 / np.sqrt(E)


def build_bass(reps=1):
    import concourse.bass as bass
    import concourse.mybir as mybir
    from concourse import bacc
    from concourse import library_config
    from concourse.tile import TileContext

    F32 = mybir.dt.float32
    BF16 = mybir.dt.bfloat16
    F8 = mybir.dt.float8e4
    DRow = mybir.MatmulPerfMode.DoubleRow
    Exp = mybir.ActivationFunctionType.Exp

    nc = bacc.Bacc()

    U8 = mybir.dt.uint8
    xT_e = nc.declare_dram_parameter("xT", [E, S], BF16, isOutput=False)
    # fp8 operands shipped as uint8 bytes (axon PJRT lacks fp8 buffers):
    # x8: [epair, 128, t, S]; wq8/wk8: [epair, 128, t, eo, 128], W pre-scaled
    # by 32 (the 1/32 is folded into the cs/sn RoPE tables).
    x8_e = nc.declare_dram_parameter("x8", [4, 128, 2, S], U8, isOutput=False)
    wq8_e = nc.declare_dram_parameter(
        "wq8", [4, 128, 2, 2, 128], U8, isOutput=False)
    wk8_e = nc.declare_dram_parameter(
        "wk8", [4, 128, 2, 2, 128], U8, isOutput=False)
    wv_e = nc.declare_dram_parameter("wv", [E, 256], BF16, isOutput=False)
    wo_e = nc.declare_dram_parameter("wo", [256, E], BF16, isOutput=False)
    cs_e = nc.declare_dram_parameter("cs", [128, S], BF16, isOutput=False)
    sn_e = nc.declare_dram_parameter("sn", [128, S], BF16, isOutput=False)
    dm_e = nc.declare_dram_parameter("dmask", [4, 128, 512], BF16, isOutput=False)
    z_e = nc.declare_dram_parameter("z", [128, 1024], U8, isOutput=False)
    out_e = nc.declare_dram_parameter("out", [S, E], BF16, isOutput=True)

    nc.gpsimd.load_library(library_config.attn)

    with TileContext(nc) as tc:
        with (
            tc.tile_pool(name="wpool", bufs=1) as wpool,
            tc.tile_pool(name="xpool", bufs=1) as xpool,
            tc.tile_pool(name="qk", bufs=1) as qkpool,
            tc.tile_pool(name="kz", bufs=1) as kzpool,
            tc.tile_pool(name="vpool", bufs=1) as vpool,
            tc.tile_pool(name="epool", bufs=8) as epool,
            tc.tile_pool(name="rtmp", bufs=4) as rtmp,
            tc.tile_pool(name="atp", bufs=1) as atpool,
            tc.tile_pool(name="ypool", bufs=4) as ypool,
            tc.tile_pool(name="npool", bufs=4) as npool,
            tc.tile_pool(name="psA", bufs=2, space="PSUM") as psA,
            tc.tile_pool(name="psS", bufs=2, space="PSUM") as psS,
            tc.tile_pool(name="psO", bufs=1, space="PSUM") as psO,
        ):
          # zero-padded per-head K stationaries: [128, 2, 512] with only rows
          # 32h..32h+31 nonzero, so scores run as full-array (untiled)
          # DoubleRow matmuls. Zero-filled ONCE (outside the rep loop: the
          # zero rows are never overwritten); head rows are DMA-copied in
          # each rep after RoPE produces the compact k8.
          k8z_t = [[None] * NQ for _ in range(HPC)]
          for jc in range(NQ):
              for h in range(HPC):
                  kz = kzpool.tile([128, 2, 512], U8, tag=f"kz{h}_{jc}",
                                   name=f"kz{h}_{jc}")
                  nc.scalar.dma_start(
                      kz[:], z_e.rearrange("p (t c) -> p t c", t=2))
                  k8z_t[h][jc] = kz

          for _rep in range(reps):
            # ---- static inputs (x first: it gates the projection matmuls;
            # split per (e-chunk, 512-col slice) and loaded j-major so the
            # first projection group starts after ~1MB instead of 4MB) ----
            x_t = [[None] * NQ for _ in range(NE)]
            x8_t = [[None] * NQ for _ in range(4)]
            for j in range(NQ):
                for pr in range(4):
                    x8t = xpool.tile([128, 2, 512], U8, tag=f"x8_{pr}_{j}",
                                     name=f"x8_{pr}_{j}")
                    nc.sync.dma_start(
                        x8t[:], x8_e[pr, :, :, 512 * j : 512 * (j + 1)])
                    x8_t[pr][j] = x8t
                for e in range(NE):
                    xt = xpool.tile([128, 512], BF16, tag=f"x{e}_{j}",
                                    name=f"x{e}_{j}")
                    nc.sync.dma_start(
                        xt[:], xT_e[128 * e : 128 * (e + 1),
                                    512 * j : 512 * (j + 1)])
                    x_t[e][j] = xt

            wq_t, wk_t, wv_t = [], [], []
            for pr in range(4):
                wqt = wpool.tile([128, 2, 2, 128], U8, tag=f"wq{pr}",
                                 name=f"wq{pr}")
                nc.sync.dma_start(wqt[:], wq8_e[pr])
                wq_t.append(wqt)
                wkt = wpool.tile([128, 2, 2, 128], U8, tag=f"wk{pr}",
                                 name=f"wk{pr}")
                nc.sync.dma_start(wkt[:], wk8_e[pr])
                wk_t.append(wkt)
            for e in range(NE):
                wvt = wpool.tile([128, 256], BF16, tag=f"wv{e}", name=f"wv{e}")
                nc.sync.dma_start(
                    wvt[:], wv_e[128 * e : 128 * (e + 1)])
                wv_t.append(wvt)
            cs_sb = wpool.tile([128, S], BF16, tag="cs")
            nc.sync.dma_start(cs_sb[:], cs_e[:])
            sn_sb = wpool.tile([128, S], BF16, tag="sn")
            nc.sync.dma_start(sn_sb[:], sn_e[:])
            dm_sb = wpool.tile([128, 4, 512], BF16, tag="dm")
            nc.sync.dma_start(dm_sb[:], dm_e.rearrange("r p c -> p r c"))
            wo_sb = wpool.tile([128, 2, E], BF16, tag="wo")
            nc.sync.dma_start(wo_sb[:], wo_e.rearrange("(c p) e -> p c e", p=128))

            # ---- projections + RoPE -------------------------------------------
            # q8_t[j]: [128, 2, 512] fp8e4; rows 32h..32h+31 = head h,
            # dim1 0/1 = rotated even/odd components
            q8_t, k8_t = ([None] * NQ for _ in range(2))
            v_t = [None] * NK

            def emit_qk(w_t, j, nm, qk_list):
                sl = slice(512 * j, 512 * (j + 1))
                pe_ps = psA.tile([128, 512], F32, tag="pp", name=f"pe_{nm}{j}")
                po_ps = psA.tile([128, 512], F32, tag="pp", name=f"po_{nm}{j}")
                for pr in range(4):
                    nc.tensor.matmul(
                        pe_ps[:], w_t[pr][:, :, 0, :].bitcast(F8),
                        x8_t[pr][j][:].bitcast(F8),
                        start=(pr == 0), stop=(pr == 3), perf_mode=DRow)
                for pr in range(4):
                    nc.tensor.matmul(
                        po_ps[:], w_t[pr][:, :, 1, :].bitcast(F8),
                        x8_t[pr][j][:].bitcast(F8),
                        start=(pr == 0), stop=(pr == 3), perf_mode=DRow)
                # drain psums to bf16 once, then RoPE in 4x bf16 DVE mode;
                # rotated outputs quantize to fp8e4 in one [128,2,512] tile
                # (dim1 = even/odd) feeding DoubleRow scores matmuls.
                pe_sb = rtmp.tile([128, 512], BF16, tag="pe_sb", name="pe_sb")
                po_sb = rtmp.tile([128, 512], BF16, tag="po_sb", name="po_sb")
                nc.vector.tensor_copy(pe_sb[:], pe_ps[:])
                nc.vector.tensor_copy(po_sb[:], po_ps[:])
                t1 = rtmp.tile([128, 512], BF16, tag="t1", name="t1")
                t2 = rtmp.tile([128, 512], BF16, tag="t2", name="t2")
                t3 = rtmp.tile([128, 512], BF16, tag="t3", name="t3")
                t4 = rtmp.tile([128, 512], BF16, tag="t4", name="t4")
                nc.vector.tensor_mul(t1[:], pe_sb[:], cs_sb[:, sl])
                nc.vector.tensor_mul(t2[:], po_sb[:], sn_sb[:, sl])
                nc.vector.tensor_mul(t3[:], pe_sb[:], sn_sb[:, sl])
                nc.vector.tensor_mul(t4[:], po_sb[:], cs_sb[:, sl])
                qk = qkpool.tile([128, 2, 512], F8, tag=f"{nm}8{j}",
                                 name=f"{nm}8{j}")
                with nc.allow_low_precision(reason="fp8 q/k for scores"):
                    nc.vector.tensor_sub(qk[:, 0, :], t1[:], t2[:])
                    nc.vector.tensor_add(qk[:, 1, :], t3[:], t4[:])
                qk_list[j] = qk

            def emit_v(i):
                # V: natural [S, dims] layout, ones column per head (65 wide)
                pv = psA.tile([128, 256], F32, tag="pp", name=f"pv{i}")
                for e in range(NE):
                    nc.tensor.matmul(
                        pv[:],
                        x_t[e][i // 4][:, 128 * (i % 4) : 128 * (i % 4) + 128],
                        wv_t[e][:],
                        start=(e == 0), stop=(e == NE - 1))
                vt = vpool.tile([128, 4, 65], BF16, tag=f"v{i}", name=f"v{i}")
                nc.vector.tensor_copy(
                    vt[:, :, 0:64], pv[:].rearrange("p (h d) -> p h d", d=64))
                nc.vector.memset(vt[:, :, 64], 1.0)
                v_t[i] = vt

            # Emission order tracks the attention dependency front: the
            # jq=3 i-loop consumes (v_t[i], ke[i//4], qe[3]) from i=0, so
            # V/K slice 0 and Q slice 3 come first; later Q slices are only
            # needed when their (later) jq pass starts.
            for step in range(NQ):
                for i in range(4 * step, 4 * step + 4):
                    emit_v(i)
                emit_qk(wk_t, step, "k", k8_t)
                # per-head row copies ride the Activation HWDGE queue so they
                # don't wait behind the input loads on the sync queue
                for h in range(HPC):
                    nc.scalar.dma_start(
                        k8z_t[h][step][32 * h : 32 * h + 32],
                        k8_t[step][32 * h : 32 * h + 32].bitcast(U8))
                emit_qk(wq_t, NQ - 1 - step, "q", q8_t)

            # ---- attention -----------------------------------------------------
            # jq descending: the longest i-loop (jq=3) starts first, the
            # shortest (jq=0) forms the kernel tail. Each jq runs as two
            # head-pair passes so the scores psum pool can double-buffer
            # (bufs=2): exp(i) overlaps scores(i+1) instead of serializing
            # the whole scores->exp->AV chain through one slot.
            for jq in range(NQ - 1, -1, -1):
                nblk = 4 * jq + 4
                at_c = [atpool.tile([128, 512], BF16, tag=f"at{c}_{jq}",
                                    name=f"at{c}_{jq}") for c in range(2)]
                for hp in range(2):
                    po = [psO.tile([65, 512], F32, tag=f"o{g}",
                                   name=f"po{hp}_{g}") for g in range(2)]

                    # software pipeline: emit scores(i+1) BEFORE exp/AV(i) so
                    # the in-order PE queue runs scores(i+1) during exp(i)
                    # instead of stalling behind AV(i)'s wait on exp(i).
                    def emit_scores(i):
                        r = i - 4 * jq
                        q0 = 128 * max(r, 0)
                        jsl = slice(128 * (i % 4), 128 * (i % 4) + 128)
                        ss = psS.tile([128, 2, 512], F32, tag="ss", name="ss")
                        for g in range(2):
                            h = 2 * hp + g
                            nc.tensor.matmul(
                                ss[:, g, q0:512],
                                k8z_t[h][i // 4][:, :, jsl].bitcast(F8),
                                q8_t[jq][:, :, q0:512],
                                start=True, stop=True, perf_mode=DRow)
                        return ss

                    def emit_expav(i, ss):
                        r = i - 4 * jq
                        q0 = 128 * max(r, 0)
                        w = 512 - q0
                        et = epool.tile([128, 2, 512], BF16, tag="e")
                        nc.scalar.activation(
                            et[:, :, q0:512], ss[:, :, q0:512], Exp,
                            scale=ATTN_SCALE)
                        if r >= 0:
                            nc.vector.tensor_mul(
                                et[:, :, q0:512], et[:, :, q0:512],
                                dm_sb[:, r, None, q0:512].to_broadcast(
                                    (128, 2, w)))
                        for g in range(2):
                            h = 2 * hp + g
                            nc.tensor.matmul(
                                po[g][:, q0:512], v_t[i][:, h, :],
                                et[:, g, q0:512],
                                start=(i == 0), stop=(i == nblk - 1))

                    prev = None
                    for i in range(nblk):
                        ss = emit_scores(i)
                        if prev is not None:
                            emit_expav(i - 1, prev)
                        prev = ss
                    emit_expav(nblk - 1, prev)

                    # normalize: at = po[0:64] * (1 / po[64]) -> bf16
                    for g in range(2):
                        rt = npool.tile([1, 512], BF16, tag="rt")
                        with nc.allow_low_precision(
                                reason="softmax denom recip in bf16"):
                            nc.vector.reciprocal(rt[:], po[g][64:65, :])
                        bt = npool.tile([64, 512], BF16, tag="bt")
                        nc.gpsimd.partition_broadcast(bt[:], rt[:])
                        nc.vector.tensor_mul(
                            at_c[hp][64 * g : 64 * g + 64, :],
                            po[g][0:64, :], bt[:])

                # output projection for this q range
                for qb in range(4):
                    lsl = slice(128 * qb, 128 * qb + 128)
                    orow = 128 * (4 * jq + qb)
                    for ec in range(2):
                        esl = slice(512 * ec, 512 * (ec + 1))
                        yp = psA.tile([128, 512], F32, tag="pp")
                        for c in range(2):
                            nc.tensor.matmul(
                                yp[:], at_c[c][:, lsl], wo_sb[:, c, esl],
                                start=(c == 0), stop=(c == 1))
                        ys = ypool.tile([128, 512], BF16, tag="y")
                        nc.vector.tensor_copy(ys[:], yp[:])
                        nc.sync.dma_start(
                            out_e[orow : orow + 128, esl], ys[:])
    nc.finalize()
    return nc


def host_inputs(x, Wq, Wk, Wv, Wo):
    """Build the 8 per-core input maps (numpy, host-side shard/permute)."""
    F8 = ml_dtypes.float8_e4m3
    perm = np.concatenate([np.arange(0, D, 2), np.arange(1, D, 2)])  # evens;odds
    d2 = D // 2
    theta = 1.0 / (ROPE_BASE ** (np.arange(d2, dtype=np.float64) * 2.0 / D))
    pos = np.arange(S, dtype=np.float64)
    ang = pos[None, :] * theta[:, None]              # [32, S]
    # q/k psums carry a 32x factor (W pre-scaled into fp8 range); fold the
    # 1/32 into the RoPE tables so the rotated q/k come out at unit scale.
    cs = np.tile(np.cos(ang) / 32.0, (4, 1)).astype(ml_dtypes.bfloat16)
    sn = np.tile(np.sin(ang) / 32.0, (4, 1)).astype(ml_dtypes.bfloat16)

    dm = np.zeros((4, 128, 512), dtype=np.float32)
    k_idx = np.arange(128)[:, None]
    c_idx = np.arange(512)[None, :]
    for r in range(4):
        dm[r] = (k_idx <= c_idx - 128 * r).astype(np.float32)
    dm = dm.astype(ml_dtypes.bfloat16)

    def pack_w8(W, ecols, ocols):
        # [E, 2(eo), 128] fp8 of 32*W -> [epair, 128, t, eo, 128] bytes
        w = np.stack([W.T[:, ecols], W.T[:, ocols]], axis=1)
        w8 = (32.0 * w).astype(F8)
        w8 = w8.reshape(4, 2, 128, 2, 128).transpose(0, 2, 1, 3, 4)
        return np.ascontiguousarray(w8).view(np.uint8)

    in_maps = []
    for c in range(NCORES):
        b, g = divmod(c, HPC)
        heads = [HPC * g + t for t in range(HPC)]
        # evens chunk cols: head-major, 32 even dims each; odds chunk likewise
        ecols = np.concatenate([D * h + perm[:d2] for h in heads])
        ocols = np.concatenate([D * h + perm[d2:] for h in heads])
        vcols = np.concatenate([D * h + np.arange(D) for h in heads])
        wv = Wv.T[:, vcols]                                      # [E, 256]
        wo = Wo[:, vcols].T.astype(ml_dtypes.bfloat16)           # [256, E]
        xb = np.ascontiguousarray(x[b].T)                        # [E, S]
        x8 = xb.astype(F8).reshape(4, 2, 128, S).transpose(0, 2, 1, 3)
        in_maps.append({
            "xT": xb.astype(ml_dtypes.bfloat16),
            "x8": np.ascontiguousarray(x8).view(np.uint8),
            "wq8": pack_w8(Wq, ecols, ocols),
            "wk8": pack_w8(Wk, ecols, ocols),
            "wv": np.ascontiguousarray(wv).astype(ml_dtypes.bfloat16),
            "wo": np.ascontiguousarray(wo),
            "cs": cs, "sn": sn, "dmask": dm,
            "z": np.zeros((128, 1024), dtype=np.uint8),
        })
    return in_maps


_CACHED = {}


def kernel(x, Wq, Wk, Wv, Wo):
    from concourse.bass_utils import run_bass_kernel_spmd

    if "nc" not in _CACHED:
        _CACHED["nc"] = build_bass()
    nc = _CACHED["nc"]
    in_maps = host_inputs(
        np.asarray(x, dtype=np.float32), np.asarray(Wq, dtype=np.float32),
        np.asarray(Wk, dtype=np.float32), np.asarray(Wv, dtype=np.float32),
        np.asarray(Wo, dtype=np.float32))
    res = run_bass_kernel_spmd(nc, in_maps, core_ids=list(range(NCORES)))
    y = np.empty((B, S, E), dtype=np.float32)
    for b in range(B):
        y[b] = sum(res.results[HPC * b + g]["out"].astype(np.float32)
                   for g in range(HPC))
    return y



# revision 5
# speedup vs baseline: 1.1221x; 1.1221x over previous
"""Multi-head causal attention (B=2, S=2048, E=1024, H=16, D=64) on 8 TRN2
NeuronCores. Sharding: batch (2) x head-groups (4 heads each) -> 8 cores.
Each core computes Q/K/V projections for its 4 heads, RoPE, flash-style
causal attention, and a partial output projection (its head columns of Wo);
the host sums the 4 partials per batch.

Layout notes:
- Q/K projections run in fp8e4 DoubleRow (x fp8 unit-scale, W fp8
  pre-scaled by 32 with the 1/32 folded into the RoPE cos/sin tables):
  2x bf16 PE throughput. V/output projections stay bf16 for accuracy.
  fp8 bytes DMA as uint8 and bitcast at use (axon PJRT lacks fp8 buffers).
- Per head, dims are permuted to [evens(32); odds(32)]; RoPE is 6
  full-width [128,512] DVE ops per S-chunk, and the rotated q/k quantize
  into one [128, 2, 512] fp8 tile per chunk (dim1 = even/odd).
- Scores: one full-array fp8 DoubleRow matmul per (head, k-block). The
  stationary is a zero-padded per-head K copy ([128,2,128], only rows
  32h..32h+31 nonzero; zeros DMA'd once on the Act HWDGE queue, head rows
  DMA-copied per rep on the gpsimd SWDGE queue so neither the input loads
  nor the exp ops queue behind them). Even/odd halves ride the DR 2-ktile
  dim. This avoids the ~2.3x per-column penalty of sub-128-row
  (tile_position) matmuls and keeps the PE in one tiling mode throughout.
- The causal mask for diagonal blocks is ADDITIVE, accumulated into the
  scores psum by an identity-stationary matmul of a {0,-960} tile before
  exp (exp underflows masked entries to ~1e-13), so the exp->AV chain has
  no DVE hop.
- Scores are computed transposed (scoresT[k,q]) so the AV matmul can use
  V as the stationary operand in natural [S, dims] layout; a ones column
  appended to V yields the softmax denominators in psum row 64.
- Softmax normalization: reciprocal (DVE) -> partition_broadcast (GPSIMD,
  attn ucode library) -> multiply during the psum->SBUF drain (DVE).
- The attention i-loop is software-pipelined (scores(i+1) emitted before
  exp/AV(i)) so the in-order PE queue runs scores under the scalar
  engine's exp. The scalar engine (~450ns/op + ~0.63ns/col from psum over
  8.9M causal exp elements) is the binding engine at ~80us/core; PE work
  is also ~80us/core. Output partials are written bf16 (halved out-DMA,
  2x DVE drain rate); the host sums partials in fp32. Measured
  ~100-115us/core on TRN2 silicon (reps=9 in-NEFF marginal timing),
  rel err ~1.6e-2 vs the 2e-2 gate (fp8 q/k quantization dominates).
"""

import sys

if "/opt/trn_rl_repo" not in sys.path:
    sys.path.insert(0, "/opt/trn_rl_repo")

import numpy as np
import ml_dtypes

B, S, E, H = 2, 2048, 1024, 16
D = E // H          # 64
HPC = 4             # heads per core
NCORES = 8
NE = E // 128       # 8 contraction chunks
NQ = S // 512       # 4 q-chunks
NK = S // 128       # 16 k-blocks
ROPE_BASE = 10000.0
ATTN_SCALE = 1.0 / np.sqrt(E)


def build_bass(reps=1):
    import concourse.bass as bass
    import concourse.mybir as mybir
    from concourse import bacc
    from concourse import library_config
    from concourse.tile import TileContext

    F32 = mybir.dt.float32
    BF16 = mybir.dt.bfloat16
    F8 = mybir.dt.float8e4
    DRow = mybir.MatmulPerfMode.DoubleRow
    Exp = mybir.ActivationFunctionType.Exp

    nc = bacc.Bacc()

    U8 = mybir.dt.uint8
    xT_e = nc.declare_dram_parameter("xT", [E, S], BF16, isOutput=False)
    # fp8 operands shipped as uint8 bytes (axon PJRT lacks fp8 buffers):
    # x8: [epair, 128, t, S]; wq8/wk8: [epair, 128, t, eo, 128], W pre-scaled
    # by 32 (the 1/32 is folded into the cs/sn RoPE tables).
    x8_e = nc.declare_dram_parameter("x8", [4, 128, 2, S], U8, isOutput=False)
    wq8_e = nc.declare_dram_parameter(
        "wq8", [4, 128, 2, 2, 128], U8, isOutput=False)
    wk8_e = nc.declare_dram_parameter(
        "wk8", [4, 128, 2, 2, 128], U8, isOutput=False)
    wv_e = nc.declare_dram_parameter("wv", [E, 256], BF16, isOutput=False)
    wo_e = nc.declare_dram_parameter("wo", [256, E], BF16, isOutput=False)
    cs_e = nc.declare_dram_parameter("cs", [128, S], BF16, isOutput=False)
    sn_e = nc.declare_dram_parameter("sn", [128, S], BF16, isOutput=False)
    dm_e = nc.declare_dram_parameter("dmask", [4, 128, 512], BF16, isOutput=False)
    id_e = nc.declare_dram_parameter("ident", [128, 128], BF16, isOutput=False)
    z_e = nc.declare_dram_parameter("z", [128, 1024], U8, isOutput=False)
    out_e = nc.declare_dram_parameter("out", [S, E], BF16, isOutput=True)

    nc.gpsimd.load_library(library_config.attn)

    with TileContext(nc) as tc:
        with (
            tc.tile_pool(name="wpool", bufs=1) as wpool,
            tc.tile_pool(name="xpool", bufs=1) as xpool,
            tc.tile_pool(name="qk", bufs=1) as qkpool,
            tc.tile_pool(name="kz", bufs=1) as kzpool,
            tc.tile_pool(name="vpool", bufs=1) as vpool,
            tc.tile_pool(name="epool", bufs=8) as epool,
            tc.tile_pool(name="rtmp", bufs=4) as rtmp,
            tc.tile_pool(name="atp", bufs=1) as atpool,
            tc.tile_pool(name="ypool", bufs=4) as ypool,
            tc.tile_pool(name="npool", bufs=4) as npool,
            tc.tile_pool(name="psA", bufs=2, space="PSUM") as psA,
            tc.tile_pool(name="psS", bufs=2, space="PSUM") as psS,
            tc.tile_pool(name="psO", bufs=1, space="PSUM") as psO,
        ):
          # zero-padded per-head K stationaries: [128, 2, 512] with only rows
          # 32h..32h+31 nonzero, so scores run as full-array (untiled)
          # DoubleRow matmuls. Zero-filled ONCE (outside the rep loop: the
          # zero rows are never overwritten); head rows are DMA-copied in
          # each rep after RoPE produces the compact k8.
          k8z_t = [[None] * NQ for _ in range(HPC)]
          for jc in range(NQ):
              for h in range(HPC):
                  kz = kzpool.tile([128, 2, 512], U8, tag=f"kz{h}_{jc}",
                                   name=f"kz{h}_{jc}")
                  nc.scalar.dma_start(
                      kz[:], z_e.rearrange("p (t c) -> p t c", t=2))
                  k8z_t[h][jc] = kz

          for _rep in range(reps):
            # ---- static inputs (x first: it gates the projection matmuls;
            # split per (e-chunk, 512-col slice) and loaded j-major so the
            # first projection group starts after ~1MB instead of 4MB) ----
            x_t = [[None] * NQ for _ in range(NE)]
            x8_t = [[None] * NQ for _ in range(4)]
            for j in range(NQ):
                for pr in range(4):
                    x8t = xpool.tile([128, 2, 512], U8, tag=f"x8_{pr}_{j}",
                                     name=f"x8_{pr}_{j}")
                    nc.sync.dma_start(
                        x8t[:], x8_e[pr, :, :, 512 * j : 512 * (j + 1)])
                    x8_t[pr][j] = x8t
                for e in range(NE):
                    xt = xpool.tile([128, 512], BF16, tag=f"x{e}_{j}",
                                    name=f"x{e}_{j}")
                    nc.sync.dma_start(
                        xt[:], xT_e[128 * e : 128 * (e + 1),
                                    512 * j : 512 * (j + 1)])
                    x_t[e][j] = xt

            wq_t, wk_t, wv_t = [], [], []
            for pr in range(4):
                wqt = wpool.tile([128, 2, 2, 128], U8, tag=f"wq{pr}",
                                 name=f"wq{pr}")
                nc.sync.dma_start(wqt[:], wq8_e[pr])
                wq_t.append(wqt)
                wkt = wpool.tile([128, 2, 2, 128], U8, tag=f"wk{pr}",
                                 name=f"wk{pr}")
                nc.sync.dma_start(wkt[:], wk8_e[pr])
                wk_t.append(wkt)
            for e in range(NE):
                wvt = wpool.tile([128, 256], BF16, tag=f"wv{e}", name=f"wv{e}")
                nc.sync.dma_start(
                    wvt[:], wv_e[128 * e : 128 * (e + 1)])
                wv_t.append(wvt)
            cs_sb = wpool.tile([128, S], BF16, tag="cs")
            nc.sync.dma_start(cs_sb[:], cs_e[:])
            sn_sb = wpool.tile([128, S], BF16, tag="sn")
            nc.sync.dma_start(sn_sb[:], sn_e[:])
            dm_sb = wpool.tile([128, 4, 512], BF16, tag="dm")
            nc.sync.dma_start(dm_sb[:], dm_e.rearrange("r p c -> p r c"))
            id_sb = wpool.tile([128, 128], BF16, tag="id")
            nc.sync.dma_start(id_sb[:], id_e[:])
            wo_sb = wpool.tile([128, 2, E], BF16, tag="wo")
            nc.sync.dma_start(wo_sb[:], wo_e.rearrange("(c p) e -> p c e", p=128))

            # ---- projections + RoPE -------------------------------------------
            # q8_t[j]: [128, 2, 512] fp8e4; rows 32h..32h+31 = head h,
            # dim1 0/1 = rotated even/odd components
            q8_t, k8_t = ([None] * NQ for _ in range(2))
            v_t = [None] * NK

            def emit_qk(w_t, j, nm, qk_list):
                sl = slice(512 * j, 512 * (j + 1))
                pe_ps = psA.tile([128, 512], F32, tag="pp", name=f"pe_{nm}{j}")
                po_ps = psA.tile([128, 512], F32, tag="pp", name=f"po_{nm}{j}")
                for pr in range(4):
                    nc.tensor.matmul(
                        pe_ps[:], w_t[pr][:, :, 0, :].bitcast(F8),
                        x8_t[pr][j][:].bitcast(F8),
                        start=(pr == 0), stop=(pr == 3), perf_mode=DRow)
                for pr in range(4):
                    nc.tensor.matmul(
                        po_ps[:], w_t[pr][:, :, 1, :].bitcast(F8),
                        x8_t[pr][j][:].bitcast(F8),
                        start=(pr == 0), stop=(pr == 3), perf_mode=DRow)
                # drain psums to bf16 once, then RoPE in 4x bf16 DVE mode;
                # rotated outputs quantize to fp8e4 in one [128,2,512] tile
                # (dim1 = even/odd) feeding DoubleRow scores matmuls.
                pe_sb = rtmp.tile([128, 512], BF16, tag="pe_sb", name="pe_sb")
                po_sb = rtmp.tile([128, 512], BF16, tag="po_sb", name="po_sb")
                nc.vector.tensor_copy(pe_sb[:], pe_ps[:])
                nc.vector.tensor_copy(po_sb[:], po_ps[:])
                t1 = rtmp.tile([128, 512], BF16, tag="t1", name="t1")
                t2 = rtmp.tile([128, 512], BF16, tag="t2", name="t2")
                t3 = rtmp.tile([128, 512], BF16, tag="t3", name="t3")
                t4 = rtmp.tile([128, 512], BF16, tag="t4", name="t4")
                nc.vector.tensor_mul(t1[:], pe_sb[:], cs_sb[:, sl])
                nc.vector.tensor_mul(t2[:], po_sb[:], sn_sb[:, sl])
                nc.vector.tensor_mul(t3[:], pe_sb[:], sn_sb[:, sl])
                nc.vector.tensor_mul(t4[:], po_sb[:], cs_sb[:, sl])
                qk = qkpool.tile([128, 2, 512], F8, tag=f"{nm}8{j}",
                                 name=f"{nm}8{j}")
                with nc.allow_low_precision(reason="fp8 q/k for scores"):
                    nc.vector.tensor_sub(qk[:, 0, :], t1[:], t2[:])
                    nc.vector.tensor_add(qk[:, 1, :], t3[:], t4[:])
                qk_list[j] = qk

            def emit_v(i):
                # V: natural [S, dims] layout, ones column per head (65 wide)
                pv = psA.tile([128, 256], F32, tag="pp", name=f"pv{i}")
                for e in range(NE):
                    nc.tensor.matmul(
                        pv[:],
                        x_t[e][i // 4][:, 128 * (i % 4) : 128 * (i % 4) + 128],
                        wv_t[e][:],
                        start=(e == 0), stop=(e == NE - 1))
                vt = vpool.tile([128, 4, 65], BF16, tag=f"v{i}", name=f"v{i}")
                nc.vector.tensor_copy(
                    vt[:, :, 0:64], pv[:].rearrange("p (h d) -> p h d", d=64))
                nc.vector.memset(vt[:, :, 64], 1.0)
                v_t[i] = vt

            # Emission order tracks the attention dependency front: the
            # jq=3 i-loop consumes (v_t[i], ke[i//4], qe[3]) from i=0, so
            # V/K slice 0 and Q slice 3 come first; later Q slices are only
            # needed when their (later) jq pass starts.
            for step in range(NQ):
                for i in range(4 * step, 4 * step + 4):
                    emit_v(i)
                emit_qk(wk_t, step, "k", k8_t)
                # per-head row copies ride the (otherwise idle) gpsimd SWDGE
                # queue: the sync queue would park them behind the multi-MB
                # input loads, and the Activation queue is in-order -- a copy
                # waiting on RoPE-k(3) there would block every exp op queued
                # behind it until all projections finish.
                for h in range(HPC):
                    nc.gpsimd.dma_start(
                        k8z_t[h][step][32 * h : 32 * h + 32],
                        k8_t[step][32 * h : 32 * h + 32].bitcast(U8))
                emit_qk(wq_t, NQ - 1 - step, "q", q8_t)

            # ---- attention -----------------------------------------------------
            # jq descending: the longest i-loop (jq=3) starts first, the
            # shortest (jq=0) forms the kernel tail. Each jq runs as two
            # head-pair passes so the scores psum pool can double-buffer
            # (bufs=2): exp(i) overlaps scores(i+1) instead of serializing
            # the whole scores->exp->AV chain through one slot.
            for jq in range(NQ - 1, -1, -1):
                nblk = 4 * jq + 4
                at_c = [atpool.tile([128, 512], BF16, tag=f"at{c}_{jq}",
                                    name=f"at{c}_{jq}") for c in range(2)]
                for hp in range(2):
                    po = [psO.tile([65, 512], F32, tag=f"o{g}",
                                   name=f"po{hp}_{g}") for g in range(2)]

                    # software pipeline: emit scores(i+1) BEFORE exp/AV(i) so
                    # the in-order PE queue runs scores(i+1) during exp(i)
                    # instead of stalling behind AV(i)'s wait on exp(i).
                    def emit_scores(i):
                        r = i - 4 * jq
                        q0 = 128 * max(r, 0)
                        jsl = slice(128 * (i % 4), 128 * (i % 4) + 128)
                        ss = psS.tile([128, 2, 512], F32, tag="ss", name="ss")
                        for g in range(2):
                            h = 2 * hp + g
                            nc.tensor.matmul(
                                ss[:, g, q0:512],
                                k8z_t[h][i // 4][:, :, jsl].bitcast(F8),
                                q8_t[jq][:, :, q0:512],
                                start=True, stop=(r < 0), perf_mode=DRow)
                        if r >= 0:
                            # additive causal mask (-960 -> exp underflows to
                            # ~0): identity-stationary matmul accumulates the
                            # mask into the scores psum, keeping the
                            # exp->AV chain free of a DVE hop.
                            for g in range(2):
                                nc.tensor.matmul(
                                    ss[:, g, q0:512], id_sb[:],
                                    dm_sb[:, r, q0:512],
                                    start=False, stop=True)
                        return ss

                    def emit_expav(i, ss):
                        r = i - 4 * jq
                        q0 = 128 * max(r, 0)
                        w = 512 - q0
                        et = epool.tile([128, 2, 512], BF16, tag="e")
                        nc.scalar.activation(
                            et[:, :, q0:512], ss[:, :, q0:512], Exp,
                            scale=ATTN_SCALE)
                        for g in range(2):
                            h = 2 * hp + g
                            nc.tensor.matmul(
                                po[g][:, q0:512], v_t[i][:, h, :],
                                et[:, g, q0:512],
                                start=(i == 0), stop=(i == nblk - 1))

                    prev = None
                    for i in range(nblk):
                        ss = emit_scores(i)
                        if prev is not None:
                            emit_expav(i - 1, prev)
                        prev = ss
                    emit_expav(nblk - 1, prev)

                    # normalize: at = po[0:64] * (1 / po[64]) -> bf16
                    for g in range(2):
                        rt = npool.tile([1, 512], BF16, tag="rt")
                        with nc.allow_low_precision(
                                reason="softmax denom recip in bf16"):
                            nc.vector.reciprocal(rt[:], po[g][64:65, :])
                        bt = npool.tile([64, 512], BF16, tag="bt")
                        nc.gpsimd.partition_broadcast(bt[:], rt[:])
                        nc.vector.tensor_mul(
                            at_c[hp][64 * g : 64 * g + 64, :],
                            po[g][0:64, :], bt[:])

                # output projection for this q range
                for qb in range(4):
                    lsl = slice(128 * qb, 128 * qb + 128)
                    orow = 128 * (4 * jq + qb)
                    for ec in range(2):
                        esl = slice(512 * ec, 512 * (ec + 1))
                        yp = psA.tile([128, 512], F32, tag="pp")
                        for c in range(2):
                            nc.tensor.matmul(
                                yp[:], at_c[c][:, lsl], wo_sb[:, c, esl],
                                start=(c == 0), stop=(c == 1))
                        ys = ypool.tile([128, 512], BF16, tag="y")
                        nc.vector.tensor_copy(ys[:], yp[:])
                        nc.sync.dma_start(
                            out_e[orow : orow + 128, esl], ys[:])
    nc.finalize()
    return nc


def host_inputs(x, Wq, Wk, Wv, Wo):
    """Build the 8 per-core input maps (numpy, host-side shard/permute)."""
    F8 = ml_dtypes.float8_e4m3
    perm = np.concatenate([np.arange(0, D, 2), np.arange(1, D, 2)])  # evens;odds
    d2 = D // 2
    theta = 1.0 / (ROPE_BASE ** (np.arange(d2, dtype=np.float64) * 2.0 / D))
    pos = np.arange(S, dtype=np.float64)
    ang = pos[None, :] * theta[:, None]              # [32, S]
    # q/k psums carry a 32x factor (W pre-scaled into fp8 range); fold the
    # 1/32 into the RoPE tables so the rotated q/k come out at unit scale.
    cs = np.tile(np.cos(ang) / 32.0, (4, 1)).astype(ml_dtypes.bfloat16)
    sn = np.tile(np.sin(ang) / 32.0, (4, 1)).astype(ml_dtypes.bfloat16)

    # additive causal mask: 0 on valid (k <= q) positions, -960 on masked
    # ones (-960 * attn_scale = -30 -> exp ~ 9e-14, negligible in the AV sum)
    dm = np.zeros((4, 128, 512), dtype=np.float32)
    k_idx = np.arange(128)[:, None]
    c_idx = np.arange(512)[None, :]
    for r in range(4):
        dm[r] = np.where(k_idx <= c_idx - 128 * r, 0.0, -960.0)
    dm = dm.astype(ml_dtypes.bfloat16)

    def pack_w8(W, ecols, ocols):
        # [E, 2(eo), 128] fp8 of 32*W -> [epair, 128, t, eo, 128] bytes
        w = np.stack([W.T[:, ecols], W.T[:, ocols]], axis=1)
        w8 = (32.0 * w).astype(F8)
        w8 = w8.reshape(4, 2, 128, 2, 128).transpose(0, 2, 1, 3, 4)
        return np.ascontiguousarray(w8).view(np.uint8)

    in_maps = []
    for c in range(NCORES):
        b, g = divmod(c, HPC)
        heads = [HPC * g + t for t in range(HPC)]
        # evens chunk cols: head-major, 32 even dims each; odds chunk likewise
        ecols = np.concatenate([D * h + perm[:d2] for h in heads])
        ocols = np.concatenate([D * h + perm[d2:] for h in heads])
        vcols = np.concatenate([D * h + np.arange(D) for h in heads])
        wv = Wv.T[:, vcols]                                      # [E, 256]
        wo = Wo[:, vcols].T.astype(ml_dtypes.bfloat16)           # [256, E]
        xb = np.ascontiguousarray(x[b].T)                        # [E, S]
        x8 = xb.astype(F8).reshape(4, 2, 128, S).transpose(0, 2, 1, 3)
        in_maps.append({
            "xT": xb.astype(ml_dtypes.bfloat16),
            "x8": np.ascontiguousarray(x8).view(np.uint8),
            "wq8": pack_w8(Wq, ecols, ocols),
            "wk8": pack_w8(Wk, ecols, ocols),
            "wv": np.ascontiguousarray(wv).astype(ml_dtypes.bfloat16),
            "wo": np.ascontiguousarray(wo),
            "cs": cs, "sn": sn, "dmask": dm,
            "ident": np.eye(128, dtype=np.float32).astype(ml_dtypes.bfloat16),
            "z": np.zeros((128, 1024), dtype=np.uint8),
        })
    return in_maps


_CACHED = {}


def kernel(x, Wq, Wk, Wv, Wo):
    from concourse.bass_utils import run_bass_kernel_spmd

    if "nc" not in _CACHED:
        _CACHED["nc"] = build_bass()
    nc = _CACHED["nc"]
    in_maps = host_inputs(
        np.asarray(x, dtype=np.float32), np.asarray(Wq, dtype=np.float32),
        np.asarray(Wk, dtype=np.float32), np.asarray(Wv, dtype=np.float32),
        np.asarray(Wo, dtype=np.float32))
    res = run_bass_kernel_spmd(nc, in_maps, core_ids=list(range(NCORES)))
    y = np.empty((B, S, E), dtype=np.float32)
    for b in range(B):
        y[b] = sum(res.results[HPC * b + g]["out"].astype(np.float32)
                   for g in range(HPC))
    return y



# revision 7
# speedup vs baseline: 1.1231x; 1.0009x over previous
"""Multi-head causal attention (B=2, S=2048, E=1024, H=16, D=64) on 8 TRN2
NeuronCores. Sharding: batch (2) x head-groups (4 heads each) -> 8 cores.
Each core computes Q/K/V projections for its 4 heads, RoPE, flash-style
causal attention, and a partial output projection (its head columns of Wo);
the host sums the 4 partials per batch.

Layout notes:
- Q/K projections run in fp8e4 DoubleRow (x fp8 unit-scale, W fp8
  pre-scaled by 32 with the 1/32 folded into the RoPE cos/sin tables):
  2x bf16 PE throughput. V/output projections stay bf16 for accuracy.
  fp8 bytes DMA as uint8 and bitcast at use (axon PJRT lacks fp8 buffers).
- Per head, dims are permuted to [evens(32); odds(32)]; RoPE is 6
  full-width [128,512] DVE ops per S-chunk, and the rotated q/k quantize
  into one [128, 2, 512] fp8 tile per chunk (dim1 = even/odd).
- Scores: one full-array fp8 DoubleRow matmul per (head, k-block). The
  stationary is a zero-padded per-head K copy ([128,2,128], only rows
  32h..32h+31 nonzero; zeros DMA'd once on the Act HWDGE queue, head rows
  DMA-copied per rep on the gpsimd SWDGE queue so neither the input loads
  nor the exp ops queue behind them). Even/odd halves ride the DR 2-ktile
  dim. This avoids the ~2.3x per-column penalty of sub-128-row
  (tile_position) matmuls and keeps the PE in one tiling mode throughout.
- The causal mask for diagonal blocks is ADDITIVE, accumulated into the
  scores psum by an identity-stationary matmul of a {0,-960} tile before
  exp (masked entries underflow to ~1e-13), so the exp->AV chain has no
  DVE hop.
- Input DMAs are ordered by first use on the serial sync queue (q/k
  weights, fp8 x, RoPE tables, then bf16 x / masks / Wo), and per step
  the k/q projections are emitted before the V matmuls, so the first
  scores->exp chain starts ~8us in instead of ~24us.
- Scores are computed transposed (scoresT[k,q]) so the AV matmul can use
  V as the stationary operand in natural [S, dims] layout; a ones column
  appended to V yields the softmax denominators in psum row 64.
- Softmax normalization: reciprocal (DVE) -> partition_broadcast (GPSIMD,
  attn ucode library) -> multiply during the psum->SBUF drain (DVE).
- The attention i-loop is software-pipelined (scores(i+1) emitted before
  exp/AV(i)) so the in-order PE queue runs scores under the scalar
  engine's exp. The scalar engine (~450ns/op + ~0.63ns/col from psum over
  8.9M causal exp elements) is the binding engine at ~80us/core; PE work
  is also ~80us/core. Output partials are written bf16 (halved out-DMA,
  2x DVE drain rate); the host sums partials in fp32. Rel err ~1.6e-2
  vs the 2e-2 gate (fp8 q/k quantization dominates).
"""

import sys

if "/opt/trn_rl_repo" not in sys.path:
    sys.path.insert(0, "/opt/trn_rl_repo")

import numpy as np
import ml_dtypes

B, S, E, H = 2, 2048, 1024, 16
D = E // H          # 64
HPC = 4             # heads per core
NCORES = 8
NE = E // 128       # 8 contraction chunks
NQ = S // 512       # 4 q-chunks
NK = S // 128       # 16 k-blocks
ROPE_BASE = 10000.0
ATTN_SCALE = 1.0 / np.sqrt(E)


def build_bass(reps=1):
    import concourse.bass as bass
    import concourse.mybir as mybir
    from concourse import bacc
    from concourse import library_config
    from concourse.tile import TileContext

    F32 = mybir.dt.float32
    BF16 = mybir.dt.bfloat16
    F8 = mybir.dt.float8e4
    DRow = mybir.MatmulPerfMode.DoubleRow
    Exp = mybir.ActivationFunctionType.Exp

    nc = bacc.Bacc()

    U8 = mybir.dt.uint8
    xT_e = nc.declare_dram_parameter("xT", [E, S], BF16, isOutput=False)
    # fp8 operands shipped as uint8 bytes (axon PJRT lacks fp8 buffers):
    # x8: [epair, 128, t, S]; wq8/wk8: [epair, 128, t, eo, 128], W pre-scaled
    # by 32 (the 1/32 is folded into the cs/sn RoPE tables).
    x8_e = nc.declare_dram_parameter("x8", [4, 128, 2, S], U8, isOutput=False)
    wq8_e = nc.declare_dram_parameter(
        "wq8", [4, 128, 2, 2, 128], U8, isOutput=False)
    wk8_e = nc.declare_dram_parameter(
        "wk8", [4, 128, 2, 2, 128], U8, isOutput=False)
    wv_e = nc.declare_dram_parameter("wv", [E, 256], BF16, isOutput=False)
    wo_e = nc.declare_dram_parameter("wo", [256, E], BF16, isOutput=False)
    cs_e = nc.declare_dram_parameter("cs", [128, S], BF16, isOutput=False)
    sn_e = nc.declare_dram_parameter("sn", [128, S], BF16, isOutput=False)
    dm_e = nc.declare_dram_parameter("dmask", [4, 128, 512], BF16, isOutput=False)
    id_e = nc.declare_dram_parameter("ident", [128, 128], BF16, isOutput=False)
    z_e = nc.declare_dram_parameter("z", [128, 1024], U8, isOutput=False)
    out_e = nc.declare_dram_parameter("out", [S, E], BF16, isOutput=True)

    nc.gpsimd.load_library(library_config.attn)

    with TileContext(nc) as tc:
        with (
            tc.tile_pool(name="wpool", bufs=1) as wpool,
            tc.tile_pool(name="xpool", bufs=1) as xpool,
            tc.tile_pool(name="qk", bufs=1) as qkpool,
            tc.tile_pool(name="kz", bufs=1) as kzpool,
            tc.tile_pool(name="vpool", bufs=1) as vpool,
            tc.tile_pool(name="epool", bufs=8) as epool,
            tc.tile_pool(name="rtmp", bufs=4) as rtmp,
            tc.tile_pool(name="atp", bufs=1) as atpool,
            tc.tile_pool(name="ypool", bufs=4) as ypool,
            tc.tile_pool(name="npool", bufs=4) as npool,
            tc.tile_pool(name="psA", bufs=2, space="PSUM") as psA,
            tc.tile_pool(name="psS", bufs=2, space="PSUM") as psS,
            tc.tile_pool(name="psO", bufs=1, space="PSUM") as psO,
        ):
          # zero-padded per-head K stationaries: [128, 2, 512] with only rows
          # 32h..32h+31 nonzero, so scores run as full-array (untiled)
          # DoubleRow matmuls. Zero-filled ONCE (outside the rep loop: the
          # zero rows are never overwritten); head rows are DMA-copied in
          # each rep after RoPE produces the compact k8.
          k8z_t = [[None] * NQ for _ in range(HPC)]
          for jc in range(NQ):
              for h in range(HPC):
                  kz = kzpool.tile([128, 2, 512], U8, tag=f"kz{h}_{jc}",
                                   name=f"kz{h}_{jc}")
                  nc.scalar.dma_start(
                      kz[:], z_e.rearrange("p (t c) -> p t c", t=2))
                  k8z_t[h][jc] = kz

          for _rep in range(reps):
            # ---- static inputs, ordered by FIRST USE on the serial sync
            # queue: the q/k weights + fp8 x + RoPE tables gate the first
            # projection/RoPE/scores chain, so they go first (~4.5MB); the
            # bf16 x for V, masks, and Wo follow. (Previously cs/sn sat
            # behind ~8.5MB, stalling the first exp until ~24us.) ----
            wq_t, wk_t, wv_t = [], [], []
            for pr in range(4):
                wqt = wpool.tile([128, 2, 2, 128], U8, tag=f"wq{pr}",
                                 name=f"wq{pr}")
                nc.sync.dma_start(wqt[:], wq8_e[pr])
                wq_t.append(wqt)
                wkt = wpool.tile([128, 2, 2, 128], U8, tag=f"wk{pr}",
                                 name=f"wk{pr}")
                nc.sync.dma_start(wkt[:], wk8_e[pr])
                wk_t.append(wkt)
            x_t = [[None] * NQ for _ in range(NE)]
            x8_t = [[None] * NQ for _ in range(4)]
            for j in [0, 3, 1, 2]:  # step-0 consumes x8[0] (k) and x8[3] (q)
                for pr in range(4):
                    x8t = xpool.tile([128, 2, 512], U8, tag=f"x8_{pr}_{j}",
                                     name=f"x8_{pr}_{j}")
                    nc.sync.dma_start(
                        x8t[:], x8_e[pr, :, :, 512 * j : 512 * (j + 1)])
                    x8_t[pr][j] = x8t
            cs_sb = wpool.tile([128, S], BF16, tag="cs")
            nc.sync.dma_start(cs_sb[:], cs_e[:])
            sn_sb = wpool.tile([128, S], BF16, tag="sn")
            nc.sync.dma_start(sn_sb[:], sn_e[:])
            for e in range(NE):
                wvt = wpool.tile([128, 256], BF16, tag=f"wv{e}", name=f"wv{e}")
                nc.sync.dma_start(
                    wvt[:], wv_e[128 * e : 128 * (e + 1)])
                wv_t.append(wvt)
            for j in range(NQ):
                for e in range(NE):
                    xt = xpool.tile([128, 512], BF16, tag=f"x{e}_{j}",
                                    name=f"x{e}_{j}")
                    nc.sync.dma_start(
                        xt[:], xT_e[128 * e : 128 * (e + 1),
                                    512 * j : 512 * (j + 1)])
                    x_t[e][j] = xt
            dm_sb = wpool.tile([128, 4, 512], BF16, tag="dm")
            nc.sync.dma_start(dm_sb[:], dm_e.rearrange("r p c -> p r c"))
            id_sb = wpool.tile([128, 128], BF16, tag="id")
            nc.sync.dma_start(id_sb[:], id_e[:])
            wo_sb = wpool.tile([128, 2, E], BF16, tag="wo")
            nc.sync.dma_start(wo_sb[:], wo_e.rearrange("(c p) e -> p c e", p=128))

            # ---- projections + RoPE -------------------------------------------
            # q8_t[j]: [128, 2, 512] fp8e4; rows 32h..32h+31 = head h,
            # dim1 0/1 = rotated even/odd components
            q8_t, k8_t = ([None] * NQ for _ in range(2))
            v_t = [None] * NK

            def emit_qk(w_t, j, nm, qk_list):
                sl = slice(512 * j, 512 * (j + 1))
                pe_ps = psA.tile([128, 512], F32, tag="pp", name=f"pe_{nm}{j}")
                po_ps = psA.tile([128, 512], F32, tag="pp", name=f"po_{nm}{j}")
                for pr in range(4):
                    nc.tensor.matmul(
                        pe_ps[:], w_t[pr][:, :, 0, :].bitcast(F8),
                        x8_t[pr][j][:].bitcast(F8),
                        start=(pr == 0), stop=(pr == 3), perf_mode=DRow)
                for pr in range(4):
                    nc.tensor.matmul(
                        po_ps[:], w_t[pr][:, :, 1, :].bitcast(F8),
                        x8_t[pr][j][:].bitcast(F8),
                        start=(pr == 0), stop=(pr == 3), perf_mode=DRow)
                # drain psums to bf16 once, then RoPE in 4x bf16 DVE mode;
                # rotated outputs quantize to fp8e4 in one [128,2,512] tile
                # (dim1 = even/odd) feeding DoubleRow scores matmuls.
                pe_sb = rtmp.tile([128, 512], BF16, tag="pe_sb", name="pe_sb")
                po_sb = rtmp.tile([128, 512], BF16, tag="po_sb", name="po_sb")
                nc.vector.tensor_copy(pe_sb[:], pe_ps[:])
                nc.vector.tensor_copy(po_sb[:], po_ps[:])
                t1 = rtmp.tile([128, 512], BF16, tag="t1", name="t1")
                t2 = rtmp.tile([128, 512], BF16, tag="t2", name="t2")
                t3 = rtmp.tile([128, 512], BF16, tag="t3", name="t3")
                t4 = rtmp.tile([128, 512], BF16, tag="t4", name="t4")
                nc.vector.tensor_mul(t1[:], pe_sb[:], cs_sb[:, sl])
                nc.vector.tensor_mul(t2[:], po_sb[:], sn_sb[:, sl])
                nc.vector.tensor_mul(t3[:], pe_sb[:], sn_sb[:, sl])
                nc.vector.tensor_mul(t4[:], po_sb[:], cs_sb[:, sl])
                qk = qkpool.tile([128, 2, 512], F8, tag=f"{nm}8{j}",
                                 name=f"{nm}8{j}")
                with nc.allow_low_precision(reason="fp8 q/k for scores"):
                    nc.vector.tensor_sub(qk[:, 0, :], t1[:], t2[:])
                    nc.vector.tensor_add(qk[:, 1, :], t3[:], t4[:])
                qk_list[j] = qk

            def emit_v(i):
                # V: natural [S, dims] layout, ones column per head (65 wide)
                pv = psA.tile([128, 256], F32, tag="pp", name=f"pv{i}")
                for e in range(NE):
                    nc.tensor.matmul(
                        pv[:],
                        x_t[e][i // 4][:, 128 * (i % 4) : 128 * (i % 4) + 128],
                        wv_t[e][:],
                        start=(e == 0), stop=(e == NE - 1))
                vt = vpool.tile([128, 4, 65], BF16, tag=f"v{i}", name=f"v{i}")
                nc.vector.tensor_copy(
                    vt[:, :, 0:64], pv[:].rearrange("p (h d) -> p h d", d=64))
                nc.vector.memset(vt[:, :, 64], 1.0)
                v_t[i] = vt

            # Emission order tracks the attention dependency front: the
            # jq=3 i-loop consumes (v_t[i], ke[i//4], qe[3]) from i=0, so
            # V/K slice 0 and Q slice 3 come first; later Q slices are only
            # needed when their (later) jq pass starts.
            for step in range(NQ):
                emit_qk(wk_t, step, "k", k8_t)
                # per-head row copies ride the (otherwise idle) gpsimd SWDGE
                # queue: the sync queue would park them behind the multi-MB
                # input loads, and the Activation queue is in-order -- a copy
                # waiting on RoPE-k(3) there would block every exp op queued
                # behind it until all projections finish.
                for h in range(HPC):
                    nc.gpsimd.dma_start(
                        k8z_t[h][step][32 * h : 32 * h + 32],
                        k8_t[step][32 * h : 32 * h + 32].bitcast(U8))
                emit_qk(wq_t, NQ - 1 - step, "q", q8_t)
                for i in range(4 * step, 4 * step + 4):
                    emit_v(i)

            # ---- attention -----------------------------------------------------
            # jq descending: the longest i-loop (jq=3) starts first, the
            # shortest (jq=0) forms the kernel tail. Each jq runs as two
            # head-pair passes so the scores psum pool can double-buffer
            # (bufs=2): exp(i) overlaps scores(i+1) instead of serializing
            # the whole scores->exp->AV chain through one slot.
            for jq in range(NQ - 1, -1, -1):
                nblk = 4 * jq + 4
                at_c = [atpool.tile([128, 512], BF16, tag=f"at{c}_{jq}",
                                    name=f"at{c}_{jq}") for c in range(2)]
                for hp in range(2):
                    po = [psO.tile([65, 512], F32, tag=f"o{g}",
                                   name=f"po{hp}_{g}") for g in range(2)]

                    # software pipeline: emit scores(i+1) BEFORE exp/AV(i) so
                    # the in-order PE queue runs scores(i+1) during exp(i)
                    # instead of stalling behind AV(i)'s wait on exp(i).
                    def emit_scores(i):
                        r = i - 4 * jq
                        q0 = 128 * max(r, 0)
                        jsl = slice(128 * (i % 4), 128 * (i % 4) + 128)
                        ss = psS.tile([128, 2, 512], F32, tag="ss", name="ss")
                        for g in range(2):
                            h = 2 * hp + g
                            nc.tensor.matmul(
                                ss[:, g, q0:512],
                                k8z_t[h][i // 4][:, :, jsl].bitcast(F8),
                                q8_t[jq][:, :, q0:512],
                                start=True, stop=(r < 0), perf_mode=DRow)
                        if r >= 0:
                            # additive causal mask (-960 -> exp underflows to
                            # ~0): identity-stationary matmul accumulates the
                            # mask into the scores psum, keeping the
                            # exp->AV chain free of a DVE hop.
                            for g in range(2):
                                nc.tensor.matmul(
                                    ss[:, g, q0:512], id_sb[:],
                                    dm_sb[:, r, q0:512],
                                    start=False, stop=True)
                        return ss

                    def emit_expav(i, ss):
                        r = i - 4 * jq
                        q0 = 128 * max(r, 0)
                        w = 512 - q0
                        et = epool.tile([128, 2, 512], BF16, tag="e")
                        nc.scalar.activation(
                            et[:, :, q0:512], ss[:, :, q0:512], Exp,
                            scale=ATTN_SCALE)
                        for g in range(2):
                            h = 2 * hp + g
                            nc.tensor.matmul(
                                po[g][:, q0:512], v_t[i][:, h, :],
                                et[:, g, q0:512],
                                start=(i == 0), stop=(i == nblk - 1))

                    prev = None
                    for i in range(nblk):
                        ss = emit_scores(i)
                        if prev is not None:
                            emit_expav(i - 1, prev)
                        prev = ss
                    emit_expav(nblk - 1, prev)

                    # normalize: at = po[0:64] * (1 / po[64]) -> bf16
                    for g in range(2):
                        rt = npool.tile([1, 512], BF16, tag="rt")
                        with nc.allow_low_precision(
                                reason="softmax denom recip in bf16"):
                            nc.vector.reciprocal(rt[:], po[g][64:65, :])
                        bt = npool.tile([64, 512], BF16, tag="bt")
                        nc.gpsimd.partition_broadcast(bt[:], rt[:])
                        nc.vector.tensor_mul(
                            at_c[hp][64 * g : 64 * g + 64, :],
                            po[g][0:64, :], bt[:])

                # output projection for this q range
                for qb in range(4):
                    lsl = slice(128 * qb, 128 * qb + 128)
                    orow = 128 * (4 * jq + qb)
                    for ec in range(2):
                        esl = slice(512 * ec, 512 * (ec + 1))
                        yp = psA.tile([128, 512], F32, tag="pp")
                        for c in range(2):
                            nc.tensor.matmul(
                                yp[:], at_c[c][:, lsl], wo_sb[:, c, esl],
                                start=(c == 0), stop=(c == 1))
                        ys = ypool.tile([128, 512], BF16, tag="y")
                        nc.vector.tensor_copy(ys[:], yp[:])
                        nc.sync.dma_start(
                            out_e[orow : orow + 128, esl], ys[:])
    nc.finalize()
    return nc


def host_inputs(x, Wq, Wk, Wv, Wo):
    """Build the 8 per-core input maps (numpy, host-side shard/permute)."""
    F8 = ml_dtypes.float8_e4m3
    perm = np.concatenate([np.arange(0, D, 2), np.arange(1, D, 2)])  # evens;odds
    d2 = D // 2
    theta = 1.0 / (ROPE_BASE ** (np.arange(d2, dtype=np.float64) * 2.0 / D))
    pos = np.arange(S, dtype=np.float64)
    ang = pos[None, :] * theta[:, None]              # [32, S]
    # q/k psums carry a 32x factor (W pre-scaled into fp8 range); fold the
    # 1/32 into the RoPE tables so the rotated q/k come out at unit scale.
    cs = np.tile(np.cos(ang) / 32.0, (4, 1)).astype(ml_dtypes.bfloat16)
    sn = np.tile(np.sin(ang) / 32.0, (4, 1)).astype(ml_dtypes.bfloat16)

    # additive causal mask: 0 on valid (k <= q) positions, -960 on masked
    # ones (-960 * attn_scale = -30 -> exp ~ 9e-14, negligible in the AV sum)
    dm = np.zeros((4, 128, 512), dtype=np.float32)
    k_idx = np.arange(128)[:, None]
    c_idx = np.arange(512)[None, :]
    for r in range(4):
        dm[r] = np.where(k_idx <= c_idx - 128 * r, 0.0, -960.0)
    dm = dm.astype(ml_dtypes.bfloat16)

    def pack_w8(W, ecols, ocols):
        # [E, 2(eo), 128] fp8 of 32*W -> [epair, 128, t, eo, 128] bytes
        w = np.stack([W.T[:, ecols], W.T[:, ocols]], axis=1)
        w8 = (32.0 * w).astype(F8)
        w8 = w8.reshape(4, 2, 128, 2, 128).transpose(0, 2, 1, 3, 4)
        return np.ascontiguousarray(w8).view(np.uint8)

    in_maps = []
    for c in range(NCORES):
        b, g = divmod(c, HPC)
        heads = [HPC * g + t for t in range(HPC)]
        # evens chunk cols: head-major, 32 even dims each; odds chunk likewise
        ecols = np.concatenate([D * h + perm[:d2] for h in heads])
        ocols = np.concatenate([D * h + perm[d2:] for h in heads])
        vcols = np.concatenate([D * h + np.arange(D) for h in heads])
        wv = Wv.T[:, vcols]                                      # [E, 256]
        wo = Wo[:, vcols].T.astype(ml_dtypes.bfloat16)           # [256, E]
        xb = np.ascontiguousarray(x[b].T)                        # [E, S]
        x8 = xb.astype(F8).reshape(4, 2, 128, S).transpose(0, 2, 1, 3)
        in_maps.append({
            "xT": xb.astype(ml_dtypes.bfloat16),
            "x8": np.ascontiguousarray(x8).view(np.uint8),
            "wq8": pack_w8(Wq, ecols, ocols),
            "wk8": pack_w8(Wk, ecols, ocols),
            "wv": np.ascontiguousarray(wv).astype(ml_dtypes.bfloat16),
            "wo": np.ascontiguousarray(wo),
            "cs": cs, "sn": sn, "dmask": dm,
            "ident": np.eye(128, dtype=np.float32).astype(ml_dtypes.bfloat16),
            "z": np.zeros((128, 1024), dtype=np.uint8),
        })
    return in_maps


_CACHED = {}


def kernel(x, Wq, Wk, Wv, Wo):
    from concourse.bass_utils import run_bass_kernel_spmd

    if "nc" not in _CACHED:
        _CACHED["nc"] = build_bass()
    nc = _CACHED["nc"]
    in_maps = host_inputs(
        np.asarray(x, dtype=np.float32), np.asarray(Wq, dtype=np.float32),
        np.asarray(Wk, dtype=np.float32), np.asarray(Wv, dtype=np.float32),
        np.asarray(Wo, dtype=np.float32))
    res = run_bass_kernel_spmd(nc, in_maps, core_ids=list(range(NCORES)))
    y = np.empty((B, S, E), dtype=np.float32)
    for b in range(B):
        y[b] = sum(res.results[HPC * b + g]["out"].astype(np.float32)
                   for g in range(HPC))
    return y

